# revision 8
# baseline (speedup 1.0000x reference)
"""BasicGCN Trainium2 Bass kernel: 3-layer GCN + mean/max pool + linear head.

8 NeuronCores SPMD. Nodes dst-sharded (graph-aligned padded layout);
small weights replicated; per-layer AllGather of scaled features (halo);
aggregation = dma_gather(rows) + one-hot DVE masks + PE matmul accumulate;
pooled sums/maxes combined with AllReduce(add/max).
"""

import os
import sys

sys.path.insert(0, "/opt/trn_rl_repo")

import numpy as np

N = 50000
E = 800000
F = 128
H = 128
C_CLS = 10
G = 64
CORES = 8
WIN = 128
HCHUNK = 512
SLOT = 64              # pooling micro-slot (graph runs aligned to this)
IDX_SPLIT = 32768
NQ = 4


def _ceil(a, b):
    return -(-a // b)


def _build_plan(edge_index, batch_index):
    src0 = edge_index[0].astype(np.int64)
    dst0 = edge_index[1].astype(np.int64)
    bi = batch_index.astype(np.int64)

    deg = np.bincount(dst0, minlength=N).astype(np.float64) + 1.0  # + self loop
    isd_node = (1.0 / np.sqrt(deg)).astype(np.float32)
    counts = np.bincount(bi, minlength=G)
    cnt_inv = (1.0 / np.maximum(counts, 1)).astype(np.float32)

    S = N // CORES
    # graph runs per core, padded to SLOT alignment
    col_of_node = np.zeros(N, dtype=np.int64)
    core_layout = []  # per core: list of (graph, node_lo, node_hi, col_lo)
    spads = []
    for c in range(CORES):
        b = bi[c * S:(c + 1) * S]
        runs = []
        if len(b):
            bounds = np.flatnonzero(np.diff(b)) + 1
            starts = np.concatenate([[0], bounds])
            ends = np.concatenate([bounds, [len(b)]])
            col = 0
            for st, en in zip(starts, ends):
                runs.append((int(b[st]), st, en, col))
                col_of_node[c * S + st:c * S + en] = col + np.arange(en - st)
                col += _ceil(en - st, SLOT) * SLOT
            spads.append(col)
        else:
            spads.append(0)
        core_layout.append(runs)
    Spad = _ceil(max(spads), HCHUNK) * HCHUNK
    Npad = CORES * Spad
    assert Npad < 65536, Npad
    NWIN = Spad // WIN
    NSLOT = Spad // SLOT
    base_b = Npad - IDX_SPLIT

    core_of_node = np.arange(N) // S
    gcol = core_of_node * Spad + col_of_node  # padded global index

    # edges + self loops, in padded coords
    esrc = np.concatenate([gcol[src0], gcol])
    edst = np.concatenate([gcol[dst0], gcol])
    order = np.argsort(edst, kind="stable")
    esrc, edst = esrc[order], edst[order]
    core_lo = np.searchsorted(edst, np.arange(CORES) * Spad)
    core_hi = np.searchsorted(edst, (np.arange(CORES) + 1) * Spad)

    plans = []
    for c in range(CORES):
        s0, s1 = core_lo[c], core_hi[c]
        csrc = esrc[s0:s1]
        cdst = edst[s0:s1] - c * Spad
        wlo = np.searchsorted(cdst, np.arange(NWIN) * WIN)
        whi = np.searchsorted(cdst, (np.arange(NWIN) + 1) * WIN)
        wins = []
        for w in range(NWIN):
            ws, we = wlo[w], whi[w]
            wsrc = csrc[ws:we]
            wdst = (cdst[ws:we] - w * WIN).astype(np.int32)
            o = np.argsort(wsrc, kind="stable")
            wsrc, wdst = wsrc[o], wdst[o]
            k = np.searchsorted(wsrc, IDX_SPLIT)
            wins.append([(wsrc[:k].astype(np.int32), wdst[:k]),
                         ((wsrc[k:] - base_b).astype(np.int32), wdst[k:])])
        plans.append(wins)

    NIG = np.zeros((NWIN, 2), dtype=np.int64)
    for w in range(NWIN):
        for g in range(2):
            mx = max(len(plans[c][w][g][0]) for c in range(CORES))
            NIG[w, g] = max(_ceil(mx, 128) * 128, 128)

    tot_chunks = int(NIG.sum() // 128)
    idx_cols = int(NIG.sum() // 16)
    gidx = np.zeros((CORES, 128, idx_cols), dtype=np.int16)
    dstv = np.full((CORES, 128, tot_chunks), 255.0, dtype=np.float32)
    offs = np.zeros((NWIN, 2, 3), dtype=np.int64)
    col = chk = 0
    for w in range(NWIN):
        for g in range(2):
            ni = int(NIG[w, g])
            offs[w, g] = (col, chk, ni)
            col += ni // 16
            chk += ni // 128
    for c in range(CORES):
        for w in range(NWIN):
            for g in range(2):
                coff, koff, ni = offs[w, g]
                gi, gd = plans[c][w][g]
                n = len(gi)
                ivec = np.zeros(ni, dtype=np.int16)
                dvec = np.full(ni, 255.0, dtype=np.float32)
                ivec[:n] = gi
                dvec[:n] = gd
                blk = ivec.reshape(ni // 16, 16).T
                for rep in range(8):
                    gidx[c, rep * 16:(rep + 1) * 16, coff:coff + ni // 16] = blk
                dstv[c, :, koff:koff + ni // 128] = dvec.reshape(ni // 128, 128).T

    # per-core staging: isd_rep, ghost mask, slot->graph masks, xT
    isd_rep = np.zeros((CORES, Spad), dtype=np.float32)
    ghost = np.zeros((CORES, Spad), dtype=np.float32)
    gmask = np.zeros((CORES, G, NSLOT), dtype=np.float32)
    for c in range(CORES):
        for (g, st, en, cl) in core_layout[c]:
            ln = en - st
            isd_rep[c, cl:cl + ln] = isd_node[c * S + st:c * S + en]
            ghost[c, cl:cl + ln] = 1.0
            gmask[c, g, cl // SLOT:(cl // SLOT) + _ceil(ln, SLOT)] = 1.0

    return dict(Spad=Spad, Npad=Npad, NWIN=NWIN, NSLOT=NSLOT, base_b=base_b,
                NIG=NIG, offs=offs, tot_chunks=tot_chunks, idx_cols=idx_cols,
                gidx=gidx, dstv=dstv, isd_rep=isd_rep, ghost=ghost,
                gmask=gmask, cnt_inv=cnt_inv, col_of_node=col_of_node, S=S)


def _build_bass(plan, trace=False):
    import concourse.bacc as bacc
    import concourse.mybir as mybir
    from concourse.tile import TileContext

    dt = mybir.dt
    BF = dt.bfloat16
    F32 = dt.float32
    AOP = mybir.AluOpType

    Spad = plan["Spad"]
    Npad = plan["Npad"]
    NWIN = plan["NWIN"]
    NSLOT = plan["NSLOT"]
    base_b = plan["base_b"]
    offs = plan["offs"]
    NT = Spad // 128
    HC = Spad // HCHUNK

    nc = bacc.Bacc("TRN2", target_bir_lowering=False, debug=False,
                   num_devices=CORES, num_swdge_queues=NQ)

    xT = nc.dram_tensor("xT", [F, Spad], BF, kind="ExternalInput").ap()
    Ws = [nc.dram_tensor(f"W{i}", [H, H], BF, kind="ExternalInput").ap() for i in range(3)]
    Bs = [nc.dram_tensor(f"b{i}", [H, 1], F32, kind="ExternalInput").ap() for i in range(3)]
    WaT = nc.dram_tensor("WaT", [2 * H, C_CLS], F32, kind="ExternalInput").ap()
    ba_in = nc.dram_tensor("ba", [C_CLS, 1], F32, kind="ExternalInput").ap()
    isd_in = nc.dram_tensor("isd_rep", [128, Spad], F32, kind="ExternalInput").ap()
    ghost_in = nc.dram_tensor("ghost", [128, Spad], BF, kind="ExternalInput").ap()
    gmask_in = nc.dram_tensor("gmask", [128, G * NSLOT], F32, kind="ExternalInput").ap()
    cnt_in = nc.dram_tensor("cntinv_rep", [128, G], F32, kind="ExternalInput").ap()
    iota_in = nc.dram_tensor("iota", [128, 128], BF, kind="ExternalInput").ap()
    ident_in = nc.dram_tensor("ident", [128, 128], BF, kind="ExternalInput").ap()
    gidx_in = nc.dram_tensor("gidx", [128, plan["idx_cols"]], dt.int16, kind="ExternalInput").ap()
    dst_in = nc.dram_tensor("dstv", [128, plan["tot_chunks"]], BF, kind="ExternalInput").ap()

    outT_d = nc.dram_tensor("outT", [C_CLS, G], F32, kind="ExternalOutput").ap()
    meanT_d = nc.dram_tensor("meanT", [128, G], F32, kind="ExternalOutput").ap()
    maxT_d = nc.dram_tensor("maxT", [128, G], F32, kind="ExternalOutput").ap()

    with TileContext(nc) as tc:
        with tc.tile_pool(name="const", bufs=1) as cp, \
             tc.tile_pool(name="xt", bufs=2) as xtp, \
             tc.tile_pool(name="hpt", bufs=1) as hptp, \
             tc.tile_pool(name="gath", bufs=6) as gp, \
             tc.tile_pool(name="sbld", bufs=6) as sp_, \
             tc.tile_pool(name="psA", bufs=4, space="PSUM") as psA, \
             tc.tile_pool(name="psB", bufs=2, space="PSUM") as psB, \
             tc.tile_pool(name="psC", bufs=1, space="PSUM") as psC, \
             tc.tile_pool(name="dram", bufs=2, space="DRAM") as dp, \
             tc.tile_pool(name="misc", bufs=3) as mp, \
             tc.tile_pool(name="pool1", bufs=1) as p1:

            iota_t = cp.tile([128, 128], BF)
            nc.sync.dma_start(out=iota_t[:, :], in_=iota_in[:, :])
            ident_t = cp.tile([128, 128], BF)
            nc.sync.dma_start(out=ident_t[:, :], in_=ident_in[:, :])
            dst_t = cp.tile([128, plan["tot_chunks"]], BF)
            nc.sync.dma_start(out=dst_t[:, :], in_=dst_in[:, :])
            W_t, b_t = [], []
            for i in range(3):
                wt = cp.tile([H, H], BF, tag=f"w{i}")
                nc.sync.dma_start(out=wt[:, :], in_=Ws[i][:, :])
                W_t.append(wt)
                bt = cp.tile([H, 1], F32, tag=f"b{i}")
                nc.sync.dma_start(out=bt[:, :], in_=Bs[i][:, :])
                b_t.append(bt)
            waT_top = cp.tile([H, C_CLS], F32, tag="wat")
            nc.sync.dma_start(out=waT_top[:, :], in_=WaT[0:H, :])
            waT_bot = cp.tile([H, C_CLS], F32, tag="wab")
            nc.sync.dma_start(out=waT_bot[:, :], in_=WaT[H:2 * H, :])
            ba_t = cp.tile([C_CLS, 1], F32)
            nc.sync.dma_start(out=ba_t[:, :], in_=ba_in[:, :])
            cnt_t = cp.tile([128, G], F32)
            nc.sync.dma_start(out=cnt_t[:, :], in_=cnt_in[:, :])

            XT = xtp.tile([128, Spad], BF, tag="xt")
            nc.sync.dma_start(out=XT[:, :], in_=xT[:, :])

            qn = 0
            for layer in range(3):
                hpT = hptp.tile([128, Spad], BF, tag="hpt")
                for j in range(HC):
                    lo = j * HCHUNK
                    ph = psB.tile([128, HCHUNK], F32, tag="ph")
                    nc.tensor.matmul(out=ph[:, :], lhsT=W_t[layer][:, :],
                                     rhs=XT[:, lo:lo + HCHUNK], start=True, stop=True)
                    isd_sl = mp.tile([128, HCHUNK], F32, tag="isdh")
                    nc.sync.dma_start(out=isd_sl[:, :], in_=isd_in[:, lo:lo + HCHUNK])
                    nc.vector.tensor_tensor(out=hpT[:, lo:lo + HCHUNK], in0=ph[:, :],
                                            in1=isd_sl[:, :], op=AOP.mult)
                hp_nm = hptp.tile([128, NT, 128], BF, tag="hpnm")
                for t in range(NT):
                    pt = psC.tile([128, 128], BF, tag="ptr")
                    nc.tensor.transpose(out=pt[:, :], in_=hpT[:, t * 128:(t + 1) * 128],
                                        identity=ident_t[:, :])
                    nc.any.tensor_copy(out=hp_nm[:, t, :], in_=pt[:, :])
                hp_shard = dp.tile([Spad, 128], BF, tag="hps")
                nc.sync.dma_start(
                    out=hp_shard[:, :].rearrange("(t p) f -> p t f", p=128),
                    in_=hp_nm[:, :, :])
                hp_full = dp.tile([Npad, 128], BF, tag="hpf")
                nc.gpsimd.collective_compute(
                    "AllGather", AOP.bypass,
                    replica_groups=[list(range(CORES))],
                    ins=[hp_shard.opt()], outs=[hp_full.opt()],
                )

                XTn = xtp.tile([128, Spad], BF, tag="xt")
                for w in range(NWIN):
                    dlo = w * WIN
                    pw = psA.tile([128, 128], F32, tag="pw")
                    first = True
                    for g in range(2):
                        coff, koff, ni = (int(v) for v in offs[w, g])
                        nchunk = ni // 128
                        gt = gp.tile([128, nchunk, 128], BF, tag=f"g{g}")
                        base = 0 if g == 0 else base_b
                        gix = mp.tile([128, ni // 16], dt.int16, tag="gix")
                        nc.sync.dma_start(out=gix[:, :], in_=gidx_in[:, coff:coff + ni // 16])
                        nc.gpsimd.dma_gather(
                            out_ap=gt[:, :, :],
                            in_ap=hp_full[base:base + IDX_SPLIT, :],
                            idxs_ap=gix[:, :],
                            num_idxs=ni, num_idxs_reg=ni, elem_size=128,
                            queue_num=qn % NQ, single_packet=False,
                        )
                        qn += 1
                        sw = sp_.tile([128, nchunk, 128], BF, tag=f"s{g}")
                        nc.vector.tensor_tensor(
                            out=sw[:, :, :],
                            in0=iota_t[:, :].unsqueeze(1).to_broadcast([128, nchunk, 128]),
                            in1=dst_t[:, koff:koff + nchunk].unsqueeze(2).to_broadcast([128, nchunk, 128]),
                            op=AOP.is_equal)
                        for k in range(nchunk):
                            nc.tensor.matmul(out=pw[:, :], lhsT=gt[:, k, :],
                                             rhs=sw[:, k, :], start=first,
                                             stop=(g == 1 and k == nchunk - 1))
                            first = False
                    isd_w = mp.tile([128, 128], F32, tag="isdw")
                    nc.sync.dma_start(out=isd_w[:, :], in_=isd_in[:, dlo:dlo + WIN])
                    tmp = mp.tile([128, 128], F32, tag="tmp")
                    nc.vector.tensor_tensor(out=tmp[:, :], in0=pw[:, :],
                                            in1=isd_w[:, :], op=AOP.mult)
                    nc.scalar.activation(XTn[:, dlo:dlo + WIN], tmp[:, :],
                                         mybir.ActivationFunctionType.Relu,
                                         bias=b_t[layer][:, 0:1])
                XT = XTn

            # ---- pooling: ghost-mask, slot reduces, masked per-graph combine ----
            XTm = hptp.tile([128, Spad], BF, tag="hpt")
            for j in range(HC):
                lo = j * HCHUNK
                gh_sl = mp.tile([128, HCHUNK], BF, tag="ghs")
                nc.sync.dma_start(out=gh_sl[:, :], in_=ghost_in[:, lo:lo + HCHUNK])
                nc.vector.tensor_tensor(out=XTm[:, lo:lo + HCHUNK],
                                        in0=XT[:, lo:lo + HCHUNK],
                                        in1=gh_sl[:, :], op=AOP.mult)
            slotsum = p1.tile([128, NSLOT], F32, tag="ssum")
            slotmax = p1.tile([128, NSLOT], F32, tag="smax")
            nc.vector.tensor_reduce(slotsum[:, :],
                                    XTm[:, :].rearrange("p (s k) -> p s k", k=SLOT),
                                    mybir.AxisListType.X, AOP.add)
            nc.vector.tensor_reduce(slotmax[:, :],
                                    XTm[:, :].rearrange("p (s k) -> p s k", k=SLOT),
                                    mybir.AxisListType.X, AOP.max)
            sums = p1.tile([128, G], F32, tag="sums")
            maxs = p1.tile([128, G], F32, tag="maxs")
            GH = G // 2
            for half in range(2):
                glo = half * GH
                gm_sl = p1.tile([128, GH * NSLOT], F32, tag="gmsl")
                nc.sync.dma_start(out=gm_sl[:, :],
                                  in_=gmask_in[:, glo * NSLOT:(glo + GH) * NSLOT])
                mb = p1.tile([128, GH, NSLOT], F32, tag="mbig")
                nc.vector.tensor_tensor(
                    out=mb[:, :, :],
                    in0=slotsum[:, :].unsqueeze(1).to_broadcast([128, GH, NSLOT]),
                    in1=gm_sl[:, :].rearrange("p (g s) -> p g s", g=GH),
                    op=AOP.mult)
                nc.vector.tensor_reduce(sums[:, glo:glo + GH], mb[:, :, :],
                                        mybir.AxisListType.X, AOP.add)
                mb2 = p1.tile([128, GH, NSLOT], F32, tag="mbig")
                nc.vector.tensor_tensor(
                    out=mb2[:, :, :],
                    in0=slotmax[:, :].unsqueeze(1).to_broadcast([128, GH, NSLOT]),
                    in1=gm_sl[:, :].rearrange("p (g s) -> p g s", g=GH),
                    op=AOP.mult)
                nc.vector.tensor_reduce(maxs[:, glo:glo + GH], mb2[:, :, :],
                                        mybir.AxisListType.X, AOP.max)

            # ---- allreduce partial sums/maxs ----
            sb_in = dp.tile([128, G], F32, tag="cin")
            sb_out = dp.tile([128, G], F32, tag="cout")
            nc.gpsimd.dma_start(sb_in[:, :], sums[:, :])
            nc.gpsimd.collective_compute(
                "AllReduce", AOP.add, replica_groups=[list(range(CORES))],
                ins=[sb_in.opt()], outs=[sb_out.opt()])
            sumsg = p1.tile([128, G], F32, tag="sumsg")
            nc.sync.dma_start(out=sumsg[:, :], in_=sb_out[:, :])
            mb_in = dp.tile([128, G], F32, tag="cin")
            mb_out = dp.tile([128, G], F32, tag="cout")
            nc.gpsimd.dma_start(mb_in[:, :], maxs[:, :])
            nc.gpsimd.collective_compute(
                "AllReduce", AOP.max, replica_groups=[list(range(CORES))],
                ins=[mb_in.opt()], outs=[mb_out.opt()])
            maxsg = p1.tile([128, G], F32, tag="maxsg")
            nc.sync.dma_start(out=maxsg[:, :], in_=mb_out[:, :])

            meanT = p1.tile([128, G], F32, tag="meanT")
            nc.vector.tensor_tensor(out=meanT[:, :], in0=sumsg[:, :],
                                    in1=cnt_t[:, :], op=AOP.mult)
            po = psC.tile([C_CLS, G], F32, tag="po")
            nc.tensor.matmul(out=po[:, :], lhsT=waT_top[:, :], rhs=meanT[:, :],
                             start=True, stop=False)
            nc.tensor.matmul(out=po[:, :], lhsT=waT_bot[:, :], rhs=maxsg[:, :],
                             start=False, stop=True)
            outT = p1.tile([C_CLS, G], F32, tag="outT")
            nc.vector.tensor_scalar(outT[:, :], po[:, :], ba_t[:, 0:1], None, AOP.add)
            nc.sync.dma_start(out=outT_d[:, :], in_=outT[:, :])
            nc.sync.dma_start(out=meanT_d[:, :], in_=meanT[:, :])
            nc.sync.dma_start(out=maxT_d[:, :], in_=maxsg[:, :])

    nc.compile()
    return nc


_CACHE = {}


def _stage_inputs(plan, x, W0, b0, W1, b1, W2, b2, Wa, ba):
    import jax.numpy as jnp

    def bf(a):
        return np.asarray(jnp.asarray(np.asarray(a, np.float32), jnp.bfloat16))

    Spad = plan["Spad"]
    S = plan["S"]
    col = plan["col_of_node"]
    x = np.asarray(x, np.float32)
    iota = np.broadcast_to(np.arange(128, dtype=np.float32), (128, 128))
    ident = np.eye(128, dtype=np.float32)

    maps = []
    for c in range(CORES):
        xTc = np.zeros((F, Spad), np.float32)
        xTc[:, col[c * S:(c + 1) * S]] = x[c * S:(c + 1) * S].T
        m = {
            "xT": bf(xTc),
            "W0": bf(W0), "W1": bf(W1), "W2": bf(W2),
            "b0": np.asarray(b0, np.float32).reshape(H, 1),
            "b1": np.asarray(b1, np.float32).reshape(H, 1),
            "b2": np.asarray(b2, np.float32).reshape(H, 1),
            "WaT": np.asarray(Wa, np.float32),
            "ba": np.asarray(ba, np.float32).reshape(C_CLS, 1),
            "isd_rep": np.broadcast_to(plan["isd_rep"][c], (128, Spad)).copy(),
            "ghost": bf(np.broadcast_to(plan["ghost"][c], (128, Spad))),
            "gmask": np.broadcast_to(plan["gmask"][c].reshape(-1), (128, G * plan["NSLOT"])).copy(),
            "cntinv_rep": np.broadcast_to(plan["cnt_inv"], (128, G)).copy(),
            "iota": bf(iota),
            "ident": bf(ident),
            "gidx": plan["gidx"][c],
            "dstv": bf(plan["dstv"][c]),
        }
        maps.append(m)
    return maps


def kernel(x, edge_index, batch_index, W0, b0, W1, b1, W2, b2, Wa, ba,
           trace=False):
    from concourse.bass_utils import run_bass_kernel_spmd

    key = ("plan", edge_index.tobytes()[:64], int(edge_index.sum()))
    if key not in _CACHE:
        _CACHE[key] = _build_plan(np.asarray(edge_index), np.asarray(batch_index))
    plan = _CACHE[key]
    bkey = ("bass", plan["Spad"], plan["tot_chunks"])
    if bkey not in _CACHE:
        _CACHE[bkey] = _build_bass(plan)
    nc = _CACHE[bkey]

    in_maps = _stage_inputs(plan, x, W0, b0, W1, b1, W2, b2, Wa, ba)
    res = run_bass_kernel_spmd(nc, in_maps, core_ids=list(range(CORES)),
                               trace=trace)
    r0 = res.results[0]
    out = np.ascontiguousarray(r0["outT"].T.astype(np.float32))
    aggr = np.concatenate([r0["meanT"].T, r0["maxT"].T], axis=1).astype(np.float32)
    kernel.last_exec_ns = res.exec_time_ns
    return out, aggr


kernel.last_exec_ns = None


# revision 12
# speedup vs baseline: 1.1632x; 1.1632x over previous
"""BasicGCN Trainium2 Bass kernel: 3-layer GCN + mean/max pool + linear head.

8 NeuronCores SPMD. Nodes dst-sharded (graph-aligned padded layout);
small weights replicated; per-layer AllGather of scaled features (halo);
aggregation = dma_gather(rows) + one-hot DVE masks + PE matmul accumulate;
pooled sums/maxes combined with AllReduce(add/max).
"""

import os
import sys

sys.path.insert(0, "/opt/trn_rl_repo")

import numpy as np

N = 50000
E = 800000
F = 128
H = 128
C_CLS = 10
G = 64
CORES = 8
WIN = 128
HCHUNK = 512
SLOT = 64              # pooling micro-slot (graph runs aligned to this)
IDX_SPLIT = 32768
NQ = 4


def _ceil(a, b):
    return -(-a // b)


def _build_plan(edge_index, batch_index):
    src0 = edge_index[0].astype(np.int64)
    dst0 = edge_index[1].astype(np.int64)
    bi = batch_index.astype(np.int64)

    deg = np.bincount(dst0, minlength=N).astype(np.float64) + 1.0  # + self loop
    isd_node = (1.0 / np.sqrt(deg)).astype(np.float32)
    counts = np.bincount(bi, minlength=G)
    cnt_inv = (1.0 / np.maximum(counts, 1)).astype(np.float32)

    S = N // CORES
    # graph runs per core, padded to SLOT alignment
    col_of_node = np.zeros(N, dtype=np.int64)
    core_layout = []  # per core: list of (graph, node_lo, node_hi, col_lo)
    spads = []
    for c in range(CORES):
        b = bi[c * S:(c + 1) * S]
        runs = []
        if len(b):
            bounds = np.flatnonzero(np.diff(b)) + 1
            starts = np.concatenate([[0], bounds])
            ends = np.concatenate([bounds, [len(b)]])
            col = 0
            for st, en in zip(starts, ends):
                runs.append((int(b[st]), st, en, col))
                col_of_node[c * S + st:c * S + en] = col + np.arange(en - st)
                col += _ceil(en - st, SLOT) * SLOT
            spads.append(col)
        else:
            spads.append(0)
        core_layout.append(runs)
    Spad = _ceil(max(spads), HCHUNK) * HCHUNK
    Npad = CORES * Spad
    assert Npad < 65536, Npad
    NWIN = Spad // WIN
    NSLOT = Spad // SLOT
    base_b = Npad - IDX_SPLIT

    core_of_node = np.arange(N) // S
    gcol = core_of_node * Spad + col_of_node  # padded global index

    # edges in padded coords (self loops handled on-chip)
    esrc = gcol[src0]
    edst = gcol[dst0]
    order = np.argsort(edst, kind="stable")
    esrc, edst = esrc[order], edst[order]
    core_lo = np.searchsorted(edst, np.arange(CORES) * Spad)
    core_hi = np.searchsorted(edst, (np.arange(CORES) + 1) * Spad)

    plans = []
    for c in range(CORES):
        s0, s1 = core_lo[c], core_hi[c]
        csrc = esrc[s0:s1]
        cdst = edst[s0:s1] - c * Spad
        wlo = np.searchsorted(cdst, np.arange(NWIN) * WIN)
        whi = np.searchsorted(cdst, (np.arange(NWIN) + 1) * WIN)
        wins = []
        for w in range(NWIN):
            ws, we = wlo[w], whi[w]
            wsrc = csrc[ws:we]
            wdst = (cdst[ws:we] - w * WIN).astype(np.int32)
            o = np.argsort(wsrc, kind="stable")
            wsrc, wdst = wsrc[o], wdst[o]
            k = np.searchsorted(wsrc, IDX_SPLIT)
            wins.append([(wsrc[:k].astype(np.int32), wdst[:k]),
                         ((wsrc[k:] - base_b).astype(np.int32), wdst[k:])])
        plans.append(wins)

    NIG = np.zeros((NWIN, 2), dtype=np.int64)
    for w in range(NWIN):
        for g in range(2):
            mx = max(len(plans[c][w][g][0]) for c in range(CORES))
            NIG[w, g] = max(_ceil(mx, 128) * 128, 128)

    tot_chunks = int(NIG.sum() // 128)
    idx_cols = int(NIG.sum() // 16)
    gidx = np.zeros((CORES, 128, idx_cols), dtype=np.int16)
    dstv = np.full((CORES, 128, tot_chunks), 255.0, dtype=np.float32)
    offs = np.zeros((NWIN, 2, 3), dtype=np.int64)
    col = chk = 0
    GW = 2
    for w0 in range(0, NWIN, GW):
        for g in range(2):
            for w in range(w0, min(w0 + GW, NWIN)):
                ni = int(NIG[w, g])
                offs[w, g] = (col, chk, ni)
                col += ni // 16
                chk += ni // 128
    for c in range(CORES):
        for w in range(NWIN):
            for g in range(2):
                coff, koff, ni = offs[w, g]
                gi, gd = plans[c][w][g]
                n = len(gi)
                ivec = np.zeros(ni, dtype=np.int16)
                dvec = np.full(ni, 255.0, dtype=np.float32)
                ivec[:n] = gi
                dvec[:n] = gd
                blk = ivec.reshape(ni // 16, 16).T
                for rep in range(8):
                    gidx[c, rep * 16:(rep + 1) * 16, coff:coff + ni // 16] = blk
                dstv[c, :, koff:koff + ni // 128] = dvec.reshape(ni // 128, 128).T

    # per-core staging: isd_rep, ghost mask, slot->graph masks, xT
    isd_rep = np.zeros((CORES, Spad), dtype=np.float32)
    ghost = np.zeros((CORES, Spad), dtype=np.float32)
    gmask = np.zeros((CORES, G, NSLOT), dtype=np.float32)
    for c in range(CORES):
        for (g, st, en, cl) in core_layout[c]:
            ln = en - st
            isd_rep[c, cl:cl + ln] = isd_node[c * S + st:c * S + en]
            ghost[c, cl:cl + ln] = 1.0
            gmask[c, g, cl // SLOT:(cl // SLOT) + _ceil(ln, SLOT)] = 1.0

    return dict(Spad=Spad, Npad=Npad, NWIN=NWIN, NSLOT=NSLOT, base_b=base_b,
                NIG=NIG, offs=offs, tot_chunks=tot_chunks, idx_cols=idx_cols,
                gidx=gidx, dstv=dstv, isd_rep=isd_rep, ghost=ghost,
                gmask=gmask, cnt_inv=cnt_inv, col_of_node=col_of_node, S=S)


def _build_bass(plan, trace=False):
    import concourse.bacc as bacc
    import concourse.mybir as mybir
    from concourse.tile import TileContext

    dt = mybir.dt
    BF = dt.bfloat16
    F32 = dt.float32
    AOP = mybir.AluOpType

    Spad = plan["Spad"]
    Npad = plan["Npad"]
    NWIN = plan["NWIN"]
    NSLOT = plan["NSLOT"]
    base_b = plan["base_b"]
    offs = plan["offs"]
    NT = Spad // 128
    HC = Spad // HCHUNK

    nc = bacc.Bacc("TRN2", target_bir_lowering=False, debug=False,
                   num_devices=CORES, num_swdge_queues=NQ)

    xT = nc.dram_tensor("xT", [F, Spad], BF, kind="ExternalInput").ap()
    Ws = [nc.dram_tensor(f"W{i}", [H, H], BF, kind="ExternalInput").ap() for i in range(3)]
    Bs = [nc.dram_tensor(f"b{i}", [H, 1], F32, kind="ExternalInput").ap() for i in range(3)]
    WaT = nc.dram_tensor("WaT", [2 * H, C_CLS], F32, kind="ExternalInput").ap()
    ba_in = nc.dram_tensor("ba", [C_CLS, 1], F32, kind="ExternalInput").ap()
    isd_in = nc.dram_tensor("isd_rep", [128, Spad], F32, kind="ExternalInput").ap()
    ghost_in = nc.dram_tensor("ghost", [128, Spad], BF, kind="ExternalInput").ap()
    gmask_in = nc.dram_tensor("gmask", [128, G * NSLOT], F32, kind="ExternalInput").ap()
    cnt_in = nc.dram_tensor("cntinv_rep", [128, G], F32, kind="ExternalInput").ap()
    iota_in = nc.dram_tensor("iota", [128, 128], BF, kind="ExternalInput").ap()
    ident_in = nc.dram_tensor("ident", [128, 128], BF, kind="ExternalInput").ap()
    gidx_in = nc.dram_tensor("gidx", [128, plan["idx_cols"]], dt.int16, kind="ExternalInput").ap()
    dst_in = nc.dram_tensor("dstv", [128, plan["tot_chunks"]], BF, kind="ExternalInput").ap()

    outT_d = nc.dram_tensor("outT", [C_CLS, G], F32, kind="ExternalOutput").ap()
    meanT_d = nc.dram_tensor("meanT", [128, G], F32, kind="ExternalOutput").ap()
    maxT_d = nc.dram_tensor("maxT", [128, G], F32, kind="ExternalOutput").ap()

    with TileContext(nc) as tc:
        with tc.tile_pool(name="const", bufs=1) as cp, \
             tc.tile_pool(name="xt", bufs=2) as xtp, \
             tc.tile_pool(name="hpt", bufs=1) as hptp, \
             tc.tile_pool(name="gath", bufs=4) as gp, \
             tc.tile_pool(name="sbld", bufs=4) as sp_, \
             tc.tile_pool(name="psA", bufs=4, space="PSUM") as psA, \
             tc.tile_pool(name="psB", bufs=2, space="PSUM") as psB, \
             tc.tile_pool(name="psC", bufs=1, space="PSUM") as psC, \
             tc.tile_pool(name="dram", bufs=2, space="DRAM") as dp, \
             tc.tile_pool(name="misc", bufs=3) as mp, \
             tc.tile_pool(name="pool1", bufs=1) as p1:

            iota_t = cp.tile([128, 128], BF)
            nc.sync.dma_start(out=iota_t[:, :], in_=iota_in[:, :])
            ident_t = cp.tile([128, 128], BF)
            nc.sync.dma_start(out=ident_t[:, :], in_=ident_in[:, :])
            dst_t = cp.tile([128, plan["tot_chunks"]], BF)
            nc.sync.dma_start(out=dst_t[:, :], in_=dst_in[:, :])
            W_t, b_t = [], []
            for i in range(3):
                wt = cp.tile([H, H], BF, tag=f"w{i}")
                nc.sync.dma_start(out=wt[:, :], in_=Ws[i][:, :])
                W_t.append(wt)
                bt = cp.tile([H, 1], F32, tag=f"b{i}")
                nc.sync.dma_start(out=bt[:, :], in_=Bs[i][:, :])
                b_t.append(bt)
            waT_top = cp.tile([H, C_CLS], F32, tag="wat")
            nc.sync.dma_start(out=waT_top[:, :], in_=WaT[0:H, :])
            waT_bot = cp.tile([H, C_CLS], F32, tag="wab")
            nc.sync.dma_start(out=waT_bot[:, :], in_=WaT[H:2 * H, :])
            ba_t = cp.tile([C_CLS, 1], F32)
            nc.sync.dma_start(out=ba_t[:, :], in_=ba_in[:, :])
            cnt_t = cp.tile([128, G], F32)
            nc.sync.dma_start(out=cnt_t[:, :], in_=cnt_in[:, :])

            XT = xtp.tile([128, Spad], BF, tag="xt")
            nc.sync.dma_start(out=XT[:, :], in_=xT[:, :])

            qn = 0
            for layer in range(3):
                hpT = hptp.tile([128, Spad], BF, tag="hpt")
                for j in range(HC):
                    lo = j * HCHUNK
                    ph = psB.tile([128, HCHUNK], F32, tag="ph")
                    nc.tensor.matmul(out=ph[:, :], lhsT=W_t[layer][:, :],
                                     rhs=XT[:, lo:lo + HCHUNK], start=True, stop=True)
                    isd_sl = mp.tile([128, HCHUNK], F32, tag="isdh")
                    nc.sync.dma_start(out=isd_sl[:, :], in_=isd_in[:, lo:lo + HCHUNK])
                    nc.vector.tensor_tensor(out=hpT[:, lo:lo + HCHUNK], in0=ph[:, :],
                                            in1=isd_sl[:, :], op=AOP.mult)
                hp_nm = hptp.tile([128, NT, 128], BF, tag="hpnm")
                for t in range(NT):
                    pt = psC.tile([128, 128], BF, tag="ptr")
                    nc.tensor.transpose(out=pt[:, :], in_=hpT[:, t * 128:(t + 1) * 128],
                                        identity=ident_t[:, :])
                    nc.any.tensor_copy(out=hp_nm[:, t, :], in_=pt[:, :])
                hp_shard = dp.tile([Spad, 128], BF, tag="hps")
                nc.sync.dma_start(
                    out=hp_shard[:, :].rearrange("(t p) f -> p t f", p=128),
                    in_=hp_nm[:, :, :])
                hp_full = dp.tile([Npad, 128], BF, tag="hpf")
                nc.gpsimd.collective_compute(
                    "AllGather", AOP.bypass,
                    replica_groups=[list(range(CORES))],
                    ins=[hp_shard.opt()], outs=[hp_full.opt()],
                )

                XTn = xtp.tile([128, Spad], BF, tag="xt")
                GW = 2
                for w0 in range(0, NWIN, GW):
                    wins = list(range(w0, min(w0 + GW, NWIN)))
                    pws = {}
                    for w in wins:
                        pws[w] = psA.tile([128, 128], F32, tag="pw", name=f"pw{w % 4}")
                    started = {w: False for w in wins}
                    for g in range(2):
                        coff0, koff0, _ = (int(v) for v in offs[wins[0], g])
                        ni_tot = sum(int(offs[w, g][2]) for w in wins)
                        nchunk_tot = ni_tot // 128
                        gt = gp.tile([128, nchunk_tot, 128], BF, tag=f"g{g}")
                        gix = mp.tile([128, ni_tot // 16], dt.int16, tag="gix")
                        nc.sync.dma_start(out=gix[:, :], in_=gidx_in[:, coff0:coff0 + ni_tot // 16])
                        base = 0 if g == 0 else base_b
                        nc.gpsimd.dma_gather(
                            out_ap=gt[:, :, :],
                            in_ap=hp_full[base:base + IDX_SPLIT, :],
                            idxs_ap=gix[:, :],
                            num_idxs=ni_tot, num_idxs_reg=ni_tot, elem_size=128,
                            queue_num=qn % NQ, single_packet=False,
                        )
                        qn += 1
                        sw = sp_.tile([128, nchunk_tot, 128], BF, tag=f"s{g}")
                        nc.vector.tensor_tensor(
                            out=sw[:, :, :],
                            in0=iota_t[:, :].unsqueeze(1).to_broadcast([128, nchunk_tot, 128]),
                            in1=dst_t[:, koff0:koff0 + nchunk_tot].unsqueeze(2).to_broadcast([128, nchunk_tot, 128]),
                            op=AOP.is_equal)
                        kbase = 0
                        for w in wins:
                            nchunk = int(offs[w, g][2]) // 128
                            for k in range(kbase, kbase + nchunk):
                                nc.tensor.matmul(out=pws[w][:, :], lhsT=gt[:, k, :],
                                                 rhs=sw[:, k, :], start=not started[w],
                                                 stop=(g == 1 and k == kbase + nchunk - 1))
                                started[w] = True
                            kbase += nchunk
                    for w in wins:
                        dlo = w * WIN
                        isd_w = mp.tile([128, 128], F32, tag="isdw")
                        nc.sync.dma_start(out=isd_w[:, :], in_=isd_in[:, dlo:dlo + WIN])
                        selft = mp.tile([128, 128], F32, tag="selft")
                        nc.vector.tensor_tensor(out=selft[:, :], in0=hpT[:, dlo:dlo + WIN],
                                                in1=isd_w[:, :], op=AOP.mult)
                        tmp = mp.tile([128, 128], F32, tag="tmp")
                        nc.vector.tensor_tensor(out=tmp[:, :], in0=pws[w][:, :],
                                                in1=isd_w[:, :], op=AOP.mult)
                        tmp2 = mp.tile([128, 128], F32, tag="tmp2")
                        nc.vector.tensor_tensor(out=tmp2[:, :], in0=tmp[:, :],
                                                in1=selft[:, :], op=AOP.add)
                        nc.scalar.activation(XTn[:, dlo:dlo + WIN], tmp2[:, :],
                                             mybir.ActivationFunctionType.Relu,
                                             bias=b_t[layer][:, 0:1])
                XT = XTn

            # ---- pooling: ghost-mask, slot reduces, masked per-graph combine ----
            XTm = hptp.tile([128, Spad], BF, tag="hpt")
            for j in range(HC):
                lo = j * HCHUNK
                gh_sl = mp.tile([128, HCHUNK], BF, tag="ghs")
                nc.sync.dma_start(out=gh_sl[:, :], in_=ghost_in[:, lo:lo + HCHUNK])
                nc.vector.tensor_tensor(out=XTm[:, lo:lo + HCHUNK],
                                        in0=XT[:, lo:lo + HCHUNK],
                                        in1=gh_sl[:, :], op=AOP.mult)
            slotsum = p1.tile([128, NSLOT], F32, tag="ssum")
            slotmax = p1.tile([128, NSLOT], F32, tag="smax")
            nc.vector.tensor_reduce(slotsum[:, :],
                                    XTm[:, :].rearrange("p (s k) -> p s k", k=SLOT),
                                    mybir.AxisListType.X, AOP.add)
            nc.vector.tensor_reduce(slotmax[:, :],
                                    XTm[:, :].rearrange("p (s k) -> p s k", k=SLOT),
                                    mybir.AxisListType.X, AOP.max)
            sums = p1.tile([128, G], F32, tag="sums")
            maxs = p1.tile([128, G], F32, tag="maxs")
            GH = G // 2
            for half in range(2):
                glo = half * GH
                gm_sl = p1.tile([128, GH * NSLOT], F32, tag="gmsl")
                nc.sync.dma_start(out=gm_sl[:, :],
                                  in_=gmask_in[:, glo * NSLOT:(glo + GH) * NSLOT])
                mb = p1.tile([128, GH, NSLOT], F32, tag="mbig")
                nc.vector.tensor_tensor(
                    out=mb[:, :, :],
                    in0=slotsum[:, :].unsqueeze(1).to_broadcast([128, GH, NSLOT]),
                    in1=gm_sl[:, :].rearrange("p (g s) -> p g s", g=GH),
                    op=AOP.mult)
                nc.vector.tensor_reduce(sums[:, glo:glo + GH], mb[:, :, :],
                                        mybir.AxisListType.X, AOP.add)
                mb2 = p1.tile([128, GH, NSLOT], F32, tag="mbig")
                nc.vector.tensor_tensor(
                    out=mb2[:, :, :],
                    in0=slotmax[:, :].unsqueeze(1).to_broadcast([128, GH, NSLOT]),
                    in1=gm_sl[:, :].rearrange("p (g s) -> p g s", g=GH),
                    op=AOP.mult)
                nc.vector.tensor_reduce(maxs[:, glo:glo + GH], mb2[:, :, :],
                                        mybir.AxisListType.X, AOP.max)

            # ---- allreduce partial sums/maxs ----
            sb_in = dp.tile([128, G], F32, tag="cin")
            sb_out = dp.tile([128, G], F32, tag="cout")
            nc.gpsimd.dma_start(sb_in[:, :], sums[:, :])
            nc.gpsimd.collective_compute(
                "AllReduce", AOP.add, replica_groups=[list(range(CORES))],
                ins=[sb_in.opt()], outs=[sb_out.opt()])
            sumsg = p1.tile([128, G], F32, tag="sumsg")
            nc.sync.dma_start(out=sumsg[:, :], in_=sb_out[:, :])
            mb_in = dp.tile([128, G], F32, tag="cin")
            mb_out = dp.tile([128, G], F32, tag="cout")
            nc.gpsimd.dma_start(mb_in[:, :], maxs[:, :])
            nc.gpsimd.collective_compute(
                "AllReduce", AOP.max, replica_groups=[list(range(CORES))],
                ins=[mb_in.opt()], outs=[mb_out.opt()])
            maxsg = p1.tile([128, G], F32, tag="maxsg")
            nc.sync.dma_start(out=maxsg[:, :], in_=mb_out[:, :])

            meanT = p1.tile([128, G], F32, tag="meanT")
            nc.vector.tensor_tensor(out=meanT[:, :], in0=sumsg[:, :],
                                    in1=cnt_t[:, :], op=AOP.mult)
            po = psC.tile([C_CLS, G], F32, tag="po")
            nc.tensor.matmul(out=po[:, :], lhsT=waT_top[:, :], rhs=meanT[:, :],
                             start=True, stop=False)
            nc.tensor.matmul(out=po[:, :], lhsT=waT_bot[:, :], rhs=maxsg[:, :],
                             start=False, stop=True)
            outT = p1.tile([C_CLS, G], F32, tag="outT")
            nc.vector.tensor_scalar(outT[:, :], po[:, :], ba_t[:, 0:1], None, AOP.add)
            nc.sync.dma_start(out=outT_d[:, :], in_=outT[:, :])
            nc.sync.dma_start(out=meanT_d[:, :], in_=meanT[:, :])
            nc.sync.dma_start(out=maxT_d[:, :], in_=maxsg[:, :])

    nc.compile()
    return nc


_CACHE = {}


def _stage_inputs(plan, x, W0, b0, W1, b1, W2, b2, Wa, ba):
    import jax.numpy as jnp

    def bf(a):
        return np.asarray(jnp.asarray(np.asarray(a, np.float32), jnp.bfloat16))

    Spad = plan["Spad"]
    S = plan["S"]
    col = plan["col_of_node"]
    x = np.asarray(x, np.float32)
    iota = np.broadcast_to(np.arange(128, dtype=np.float32), (128, 128))
    ident = np.eye(128, dtype=np.float32)

    maps = []
    for c in range(CORES):
        xTc = np.zeros((F, Spad), np.float32)
        xTc[:, col[c * S:(c + 1) * S]] = x[c * S:(c + 1) * S].T
        m = {
            "xT": bf(xTc),
            "W0": bf(W0), "W1": bf(W1), "W2": bf(W2),
            "b0": np.asarray(b0, np.float32).reshape(H, 1),
            "b1": np.asarray(b1, np.float32).reshape(H, 1),
            "b2": np.asarray(b2, np.float32).reshape(H, 1),
            "WaT": np.asarray(Wa, np.float32),
            "ba": np.asarray(ba, np.float32).reshape(C_CLS, 1),
            "isd_rep": np.broadcast_to(plan["isd_rep"][c], (128, Spad)).copy(),
            "ghost": bf(np.broadcast_to(plan["ghost"][c], (128, Spad))),
            "gmask": np.broadcast_to(plan["gmask"][c].reshape(-1), (128, G * plan["NSLOT"])).copy(),
            "cntinv_rep": np.broadcast_to(plan["cnt_inv"], (128, G)).copy(),
            "iota": bf(iota),
            "ident": bf(ident),
            "gidx": plan["gidx"][c],
            "dstv": bf(plan["dstv"][c]),
        }
        maps.append(m)
    return maps


def kernel(x, edge_index, batch_index, W0, b0, W1, b1, W2, b2, Wa, ba,
           trace=False):
    from concourse.bass_utils import run_bass_kernel_spmd

    key = ("plan", edge_index.tobytes()[:64], int(edge_index.sum()))
    if key not in _CACHE:
        _CACHE[key] = _build_plan(np.asarray(edge_index), np.asarray(batch_index))
    plan = _CACHE[key]
    bkey = ("bass", plan["Spad"], plan["tot_chunks"])
    if bkey not in _CACHE:
        _CACHE[bkey] = _build_bass(plan)
    nc = _CACHE[bkey]

    in_maps = _stage_inputs(plan, x, W0, b0, W1, b1, W2, b2, Wa, ba)
    res = run_bass_kernel_spmd(nc, in_maps, core_ids=list(range(CORES)),
                               trace=trace)
    r0 = res.results[0]
    out = np.ascontiguousarray(r0["outT"].T.astype(np.float32))
    aggr = np.concatenate([r0["meanT"].T, r0["maxT"].T], axis=1).astype(np.float32)
    kernel.last_exec_ns = res.exec_time_ns
    return out, aggr


kernel.last_exec_ns = None
